# revision 7
# baseline (speedup 1.0000x reference)
"""GNN message-passing kernel for 8 TRN2 NeuronCores.

Patches are sharded 16 ways (2 half-shards per core). Per step each core
receives its assembled input block G (self + 3 gathered neighbour feature
groups, fp16, feature-on-partition layout), runs the MLP
(K=128 matmul -> tanh -> M=16 matmul, col-tiled over the 4 batches) and
returns the dynamic-state increment F. The host keeps the fp32 master state,
performs the neighbour gather between steps, and executes the same NEFF 4x.
"""

import os
import sys

sys.path.insert(0, "/opt/trn_rl_repo")
# self-heal if a previous crashed run left the NeuronCores wedged
os.environ.setdefault("NEURON_RT_RESET_CORES", "1")

import numpy as np

import concourse.bacc as bacc
import concourse.bass as bass
import concourse.mybir as mybir
import concourse.tile as tile
from concourse.bass_utils import run_bass_kernel_spmd

N = 81920
B = 4
DL = 32
DD = 16
H = 128
NSTEPS = 4
NCORES = 8
NHALF = 16
SH = N // NHALF  # 5120 patches per half-shard
CH = 512
NCH = SH // CH

_cache = {}
_last_exec_ns = 0


def _build_nc():
    nc = bacc.Bacc(None, target_bir_lowering=False, debug=False)
    f16, f32 = mybir.dt.float16, mybir.dt.float32
    g_in = [nc.dram_tensor(f"g{s}", [128, 4 * SH], f16, kind="ExternalInput") for s in (0, 1)]
    w1_in = nc.dram_tensor("w1p", [128, 128], f16, kind="ExternalInput")
    w2_in = nc.dram_tensor("w2z", [128, 32], f16, kind="ExternalInput")
    b1_in = nc.dram_tensor("b1v", [128, 1], f32, kind="ExternalInput")
    f_out = [nc.dram_tensor(f"f{s}", [128, SH], f32, kind="ExternalOutput") for s in (0, 1)]

    with tile.TileContext(nc) as tc:
        with (
            tc.tile_pool(name="const", bufs=1) as cpool,
            tc.tile_pool(name="gbuf", bufs=1) as gpool,
            tc.tile_pool(name="work", bufs=8) as wpool,
            tc.tile_pool(name="ps1", bufs=4, space="PSUM") as ps1pool,
            tc.tile_pool(name="ps2", bufs=2, space="PSUM") as ps2pool,
        ):
            w1t = cpool.tile([128, 128], f16, tag="w1")
            w2t = cpool.tile([128, 32], f16, tag="w2")
            b1t = cpool.tile([128, 1], f32, tag="b1")
            nc.sync.dma_start(w1t[:], w1_in[:])
            nc.sync.dma_start(w2t[:], w2_in[:])
            nc.sync.dma_start(b1t[:], b1_in[:])
            for s in (0, 1):
                # chunk-major layout: free dim = (chunk, batch, within-chunk)
                # so each 4KB-per-partition chunk DMA unblocks its own matmuls
                gtiles = []
                for ch in range(NCH):
                    gt = gpool.tile([128, 4 * CH], f16, tag=f"g{s}_{ch}")
                    nc.sync.dma_start(
                        gt[:], g_in[s][:, ch * 4 * CH : (ch + 1) * 4 * CH]
                    )
                    gtiles.append(gt)
                for ch in range(NCH):
                    g = gtiles[ch]
                    ps2 = ps2pool.tile([128, CH], f32, tag="ps2")
                    hts = []
                    for b in range(4):
                        ps1 = ps1pool.tile([128, CH], f32, tag="ps1")
                        nc.tensor.matmul(
                            ps1[:],
                            w1t[:],
                            g[:, b * CH : (b + 1) * CH],
                            start=True,
                            stop=True,
                        )
                        ht = wpool.tile([128, CH], f16, tag="h")
                        nc.scalar.activation(
                            ht[:], ps1[:], mybir.ActivationFunctionType.Tanh, bias=b1t[:]
                        )
                        hts.append(ht)
                    for b in range(4):
                        nc.tensor.matmul(
                            ps2[32 * b : 32 * b + 32, :],
                            w2t[:],
                            hts[b][:],
                            start=True,
                            stop=True,
                            tile_position=(0, 32 * b),
                        )
                    ft = wpool.tile([128, CH], f32, tag="f")
                    nc.vector.tensor_copy(ft[:], ps2[:])
                    nc.sync.dma_start(f_out[s][:, ch * CH : (ch + 1) * CH], ft[:])
    nc.compile()
    return nc


def kernel(z_old, neighbour_list, W1, b1, W2, b2):
    global _last_exec_ns
    _last_exec_ns = 0
    if "nc" not in _cache:
        _cache["nc"] = _build_nc()
    nc = _cache["nc"]
    nl = np.asarray(neighbour_list)

    w1p = np.ascontiguousarray(
        W1.reshape(DL, 4, H).transpose(1, 0, 2).reshape(128, H)
    ).astype(np.float16)
    w2z = np.zeros((H, 32), np.float16)
    w2z[:, :DD] = W2.astype(np.float16)
    b1v = np.ascontiguousarray(np.asarray(b1).reshape(H, 1)).astype(np.float32)

    z = np.array(z_old, dtype=np.float32, copy=True)  # [B, N, DL] master state
    for _step in range(NSTEPS):
        z16 = z.astype(np.float16)
        in_maps = []
        for c in range(NCORES):
            m = {"w1p": w1p, "w2z": w2z, "b1v": b1v}
            for s in (0, 1):
                h = 2 * c + s
                shard = slice(h * SH, (h + 1) * SH)
                G = np.empty((128, 4, SH), np.float16)
                G[0:32] = z16[:, shard, :].transpose(2, 0, 1)
                for j in range(3):
                    G[32 * (j + 1) : 32 * (j + 2)] = z16[:, nl[shard, j], :].transpose(2, 0, 1)
                # -> chunk-major (chunk, batch, within-chunk)
                m[f"g{s}"] = np.ascontiguousarray(
                    G.reshape(128, 4, NCH, CH).transpose(0, 2, 1, 3)
                ).reshape(128, 4 * SH)
            in_maps.append(m)
        res = run_bass_kernel_spmd(nc, in_maps, core_ids=list(range(NCORES)))
        if res.exec_time_ns:
            _last_exec_ns += res.exec_time_ns
        for c in range(NCORES):
            for s in (0, 1):
                h = 2 * c + s
                shard = slice(h * SH, (h + 1) * SH)
                f = res.results[c][f"f{s}"]  # [128, SH] fp32, rows 32b+d
                F = f.reshape(4, 32, SH)[:, :DD, :]
                z[:, shard, :DD] += F.transpose(0, 2, 1) + np.asarray(b2)[None, None, :]
    return z
